# revision 30
# baseline (speedup 1.0000x reference)
"""Trainium2 Bass kernel for nn_LocalRNN: 8-step CTRNN over sliding windows.

Math:
  For each position l: h_{k+1} = a*h_k + relu(h_k @ W* + u*[l+k]),  h_0 = 0
  where a = 1 - 1/tau, W* = W * (1/tau) (columns), u* = Xp @ W_in* + b*,
  W_in* = W_in * (1/tau), b* = b * (1/tau).  Output = h_8 per position.
  (Uses relu(c*z) = c*relu(z) for c>0 to fold 1/tau into the weights, and
  the fact that the input projection is shared across overlapping windows.)

Sharding: batch dim (8) across the 8 NeuronCores, weights replicated.
On-chip layout is transposed ([d on partitions, positions on free dim]) so
matmuls contract d on the partition axis; the host uploads x pre-transposed
in bf16 and transposes the bf16 [d, pos] output back (layout marshalling).

v2: everything bf16 (PE same speed as f32r, but DVE 2-byte fast modes +
half the DMA bytes). Per step the four [128,1024] tiles split work:
  - u-add into PSUM: identity matmul on PE for 3 tiles, one tile goes the
    DVE route (stt z+u then tensor_scalar max) to shave PE columns
  - relu: ACT for 3 tiles (PSUM->SBUF bf16)
  - h-update (h' = a*h + r): DVE tensor_scalar+tensor_tensor (4x/2x modes)
    for 2 tiles, gpsimd scalar_tensor_tensor for 2 tiles
Input x lands via 4 position-quarter DMAs on 4 queues; output leaves bf16.
"""

import numpy as np
import ml_dtypes
from contextlib import ExitStack

import concourse.bass as bass
import concourse.tile as tile
from concourse import bacc, mybir
from concourse.bass_utils import run_bass_kernel_spmd

B, L, D, KSIZE = 8, 2048, 256, 8
P = 128
NCORES = 8
MMN = 512                    # matmul moving free dim (PSUM bank limit)
WCH = 1024                   # tile width for PSUM tiles / elementwise ops
NW = L // WCH                # 2
UCOLS = L + KSIZE - 1        # 2055
PAD = KSIZE - 1              # 7
DB = D // P                  # 2 d-blocks
F32 = mybir.dt.float32
BF16 = mybir.dt.bfloat16
AF = mybir.ActivationFunctionType
ALU = mybir.AluOpType
BF16NP = ml_dtypes.bfloat16

# packed bf16 const blob: wint0|wint1|wt0|wt1|identity (one DMA, wide lines)
CW_COLS = 4 * D + P
CW_W = 2 * D
CW_ID = 4 * D
# packed f32 consts blob: bst | at | pad src
CF_COLS = 2 * DB + PAD + 1
_cache = {}

# --- tuning flags ---
DVE_SIDE_TILE = False     # one tile/step adds u on DVE instead of PE identity
N_WARM = 20               # dummy matmuls (last 10 narrow) to bridge to x-arrival
NXP = 8                   # x DMA pieces (256 cols each) across the 3 rings


def _build_program():
    nc = bacc.Bacc(
        "TRN2",
        target_bir_lowering=False,
        debug=False,
        enable_asserts=False,
        num_devices=NCORES,
    )
    # x uploaded pre-transposed bf16: (D, L), row d -> [d, positions]
    x_d = nc.dram_tensor("xt", (D, L), BF16, kind="ExternalInput").ap()
    cw_d = nc.dram_tensor("constsw", (P, CW_COLS), BF16, kind="ExternalInput").ap()
    cf_d = nc.dram_tensor("constsf", (P, CF_COLS), F32, kind="ExternalInput").ap()
    # output in T-layout bf16: (D, L); host transposes + upcasts
    out_d = nc.dram_tensor("out", (D, L), BF16, kind="ExternalOutput").ap()
    # scratch target for ring-warming writes (absorbs the ~3us wake latency
    # of a DMA ring before the real output transfer)
    scr_d = nc.dram_tensor("scr", (P, 8), BF16, kind="Internal").ap()

    with tile.TileContext(nc) as tc, ExitStack() as ctx:
        consts = ctx.enter_context(tc.tile_pool(name="consts", bufs=1))
        big = ctx.enter_context(tc.tile_pool(name="big", bufs=1))
        rp = ctx.enter_context(tc.tile_pool(name="rp", bufs=4))
        ahp = ctx.enter_context(tc.tile_pool(name="ahp", bufs=3))
        # single PSUM pool: [128,1024] slot = 2 banks, bufs=4 -> all 8 banks
        zp = ctx.enter_context(tc.tile_pool(name="zp", bufs=4, space="PSUM"))

        # --- PE warmup: dummy matmuls on zeros to engage the clock early.
        # Coarse 512-wide ones bridge most of the x DMA wait; a tail of
        # 128-wide ones keeps PE busy to the arrival with <=0.1us overshoot.
        dummy = big.tile([P, MMN], BF16, name="dummy")
        nc.vector.memset(dummy[:], 0.0)
        for w in range(N_WARM):
            warm = zp.tile([P, WCH], F32, name="warm", tag="z")
            wn = MMN if w < N_WARM - 10 else P
            nc.tensor.matmul(warm[:, 0:wn], lhsT=dummy[:, 0:P],
                             rhs=dummy[:, 0:wn], start=True, stop=True)

        # --- constants ---
        cw = consts.tile([P, CW_COLS], BF16, name="cw")
        cf = consts.tile([P, CF_COLS], F32, name="cf")
        wint = [cw[:, i * D:(i + 1) * D] for i in range(DB)]
        wt = [cw[:, CW_W + i * D:CW_W + (i + 1) * D] for i in range(DB)]
        identb = cw[:, CW_ID:CW_ID + P]
        bst = cf[:, 0:DB]
        at = cf[:, DB:2 * DB]
        padsrc = cf[:, 2 * DB:2 * DB + PAD]

        # --- persistent buffers (bf16) ---
        # x as one [p, i*L + c] tile so each DMA piece is a contiguous
        # per-partition run (>=4KB lines; small lines collapse DMA BW)
        xall = big.tile([P, DB * L], BF16, name="xall")
        ut = [big.tile([P, UCOLS], BF16, name=f"ut{i}") for i in range(DB)]
        hball = [big.tile([P, DB * L], BF16, name=f"hb{s}") for s in range(2)]
        hb = [[hball[s][:, i * L:(i + 1) * L] for i in range(DB)]
              for s in range(2)]
        h1 = hb[1]

        # --- input DMAs. Each ring has ~3us issue-to-first-packet latency
        # and throughput collapses below ~2KB per-partition lines, so: one
        # full d-block of x per ring (4KB lines, first in the ring), and all
        # weights in a single wide blob on the third ring.
        xsrc = x_d.rearrange("(i p) c -> p i c", p=P)
        nc.sync.dma_start(xall[:, 0:L], xsrc[:, 0, :])
        nc.scalar.dma_start(xall[:, L:2 * L], xsrc[:, 1, :])
        nc.gpsimd.dma_start(cf[:], cf_d[:, :])
        nc.gpsimd.dma_start(cw[:], cw_d[:, :])

        # u pad cols + h1 pad cols (also warms the ACT table early):
        # u[:, :7] = b*, h1[:, :7] = relu(b*)
        for j in range(DB):
            nc.scalar.activation(
                ut[j][:, 0:PAD], padsrc,
                AF.Identity, bias=bst[:, j:j + 1], scale=0.0,
            )
            nc.scalar.activation(
                h1[j][:, 0:PAD], padsrc,
                AF.Relu, bias=bst[:, j:j + 1], scale=0.0,
            )

        # --- u projection: 4 tiles (gw, j); i-outer order shares LDWEIGHTS
        # across the two 512 halves. Post ops split between ACT and DVE.
        for gw in range(2):
            for j in range(DB):
                zt = zp.tile([P, WCH], F32, name="zu", tag="z")
                for i in range(DB):
                    for half in range(2):
                        xs = i * L + gw * WCH + half * MMN
                        nc.tensor.matmul(
                            zt[:, half * MMN:(half + 1) * MMN],
                            lhsT=wint[i][:, j * P:(j + 1) * P],
                            rhs=xall[:, xs:xs + MMN],
                            start=(i == 0),
                            stop=(i == DB - 1),
                        )
                # u positions [7+1024gw, 7+1024(gw+1)), h1 same minus tail
                us = PAD + gw * WCH
                hw = WCH if gw == 0 else WCH - PAD
                if gw == 0:
                    nc.scalar.activation(
                        ut[j][:, us:us + WCH], zt[:],
                        AF.Identity, bias=bst[:, j:j + 1], scale=1.0,
                    )
                    nc.vector.tensor_scalar(
                        out=h1[j][:, us:us + hw], in0=zt[:, 0:hw],
                        scalar1=bst[:, j:j + 1], scalar2=0.0,
                        op0=ALU.add, op1=ALU.max,
                    )
                else:
                    nc.vector.tensor_scalar(
                        out=ut[j][:, us:us + WCH], in0=zt[:],
                        scalar1=bst[:, j:j + 1], scalar2=None,
                        op0=ALU.add,
                    )
                    nc.scalar.activation(
                        h1[j][:, us:us + hw], zt[:, 0:hw],
                        AF.Relu, bias=bst[:, j:j + 1], scale=1.0,
                    )

        # --- steps 1..7 ---
        for k in range(1, KSIZE):
            hc = hb[k % 2]
            hn = hb[(k + 1) % 2]
            tix = 0
            for c in range(NW):
                cs = c * WCH
                for j in range(DB):
                    dve_side = DVE_SIDE_TILE and (c, j) == (0, 1)
                    zt = zp.tile([P, WCH], F32, name="zt", tag="z")
                    if not dve_side:
                        # identity matmul first (u ready early), W blocks after
                        for half in range(2):
                            nc.tensor.matmul(
                                zt[:, half * MMN:(half + 1) * MMN],
                                lhsT=identb,
                                rhs=ut[j][:, k + cs + half * MMN:
                                          k + cs + half * MMN + MMN],
                                start=True, stop=False,
                            )
                    for i in range(DB):
                        last = (i == DB - 1)
                        for half in range(2):
                            hs = cs + half * MMN
                            nc.tensor.matmul(
                                zt[:, half * MMN:(half + 1) * MMN],
                                lhsT=wt[i][:, j * P:(j + 1) * P],
                                rhs=hc[i][:, hs:hs + MMN],
                                start=(dve_side and i == 0),
                                stop=last,
                            )
                    # relu + h-update (h' = a*h + r): the a-scale TS only
                    # needs the previous h so it runs early; only the TT add
                    # (0.7us) sits after the relu on the critical path. The
                    # very last chunk (step 7, c1) runs at 512 granularity to
                    # shorten the drain into the output DMA.
                    fine = (k == KSIZE - 1 and c == NW - 1)
                    nsub = 2 if fine else 1
                    sw = WCH // nsub
                    ah = ahp.tile([P, WCH], BF16, name="ah", tag="ah")
                    nc.vector.tensor_scalar(
                        out=ah[:], in0=hc[j][:, cs:cs + WCH],
                        scalar1=at[:, j:j + 1], scalar2=None,
                        op0=ALU.mult,
                    )
                    for s in range(nsub):
                        ss = s * sw
                        r = rp.tile([P, sw], BF16, name="r", tag="r")
                        if fine and j == DB - 1 and s == 0:
                            # very last tile: run this relu half on DVE so
                            # both halves drain the PSUM in parallel
                            nc.vector.tensor_scalar(
                                out=r[:], in0=zt[:, ss:ss + sw],
                                scalar1=0.0, scalar2=None, op0=ALU.max,
                            )
                        else:
                            nc.scalar.activation(
                                r[:], zt[:, ss:ss + sw], AF.Relu)
                        nc.vector.tensor_tensor(
                            hn[j][:, cs + ss:cs + ss + sw],
                            ah[:, ss:ss + sw], r[:], ALU.add,
                        )
                    tix += 1
            # wake the output rings one step ahead of the real transfers
            if k == KSIZE - 2 and c == NW - 1:
                nc.sync.dma_start(scr_d[:, 0:4], dummy[:, 0:4])
                nc.gpsimd.dma_start(scr_d[:, 4:8], dummy[:, 4:8])
            # output: one full d-block per ring (4KB dram lines; chunked
            # position pieces would drop to 1KB lines at ~1/4 the rate).
            # Block j=0 completes one update earlier, staggering the rings.
            if k == KSIZE - 1 and c == NW - 1:
                h8all = hball[(k + 1) % 2]
                osrc = out_d.rearrange("(i p) c -> p i c", p=P)
                nc.sync.dma_start(osrc[:, 0, :], h8all[:, 0:L])
                nc.gpsimd.dma_start(osrc[:, 1, :], h8all[:, L:2 * L])

    nc.compile()
    return nc


def get_program():
    if "nc" not in _cache:
        _cache["nc"] = _build_program()
    return _cache["nc"]


def make_in_maps(x, weight, input_weight, bias, tau):
    x = np.asarray(x, dtype=np.float32)
    weight = np.asarray(weight, dtype=np.float32)
    input_weight = np.asarray(input_weight, dtype=np.float32)
    bias = np.asarray(bias, dtype=np.float32).reshape(1, D)
    tau = np.asarray(tau, dtype=np.float32).reshape(1, D)

    inv_tau = 1.0 / tau                       # (1, D)
    a = 1.0 - inv_tau
    wstar = (weight * inv_tau).astype(np.float32)          # scale columns
    winstar = (input_weight * inv_tau).astype(np.float32)
    bstar = (bias * inv_tau).astype(np.float32)
    # per-partition layout (P, DB): col j holds elems [j*P, (j+1)*P)
    bstar_t = bstar.reshape(DB, P).T
    a_t = a.reshape(DB, P).T
    ident = np.eye(P, dtype=np.float32)

    cwb = np.concatenate(
        [winstar[0:P, :], winstar[P:D, :], wstar[0:P, :], wstar[P:D, :],
         ident], axis=1)
    cf = np.concatenate(
        [bstar_t, a_t, np.zeros((P, PAD + 1), np.float32)], axis=1)

    shared = {
        "constsw": np.ascontiguousarray(cwb.astype(BF16NP)),
        "constsf": np.ascontiguousarray(cf),
    }
    return [
        {"xt": np.ascontiguousarray(x[b].T.astype(BF16NP)), **shared}
        for b in range(NCORES)
    ]


def kernel(x, weight, input_weight, bias, tau, ksize, _trace=False):
    assert int(ksize) == KSIZE
    nc = get_program()
    in_maps = make_in_maps(x, weight, input_weight, bias, tau)
    res = run_bass_kernel_spmd(
        nc, in_maps, core_ids=list(range(NCORES)), trace=_trace
    )
    out = np.stack(
        [np.ascontiguousarray(res.results[b]["out"].T) for b in range(NCORES)],
        axis=0,
    )
    if _trace:
        _cache["last_results"] = res
    return out.astype(np.float32)
